# revision 16
# baseline (speedup 1.0000x reference)
"""MoE (AriaExperts) Trainium2 kernel — expert parallelism across 8 NeuronCores.

Strategy:
  - Host: top-2 routing + softmax over [2048, 8] logits (tiny), build the
    per-expert token batches (the "all-to-all" is realized at input
    distribution time), and the weighted scatter-add combine at the end.
  - Device (SPMD, 1 expert per core): dense GEMM chain in transposed
    activation layout so both matmuls consume the expert weights directly
    as the stationary (lhsT) operand with zero on-device transposes:
        H^T  = W1^T-tiles @ X^T      [2*INTER, C]
        actT = silu(projT) * gateT   [INTER, C]
        outT = W2-tiles   @ actT     [HIDDEN, C]
    All matmul dtypes are 1-cycle/row; f32 PSUM accumulation.

  Trace-driven design (bf16 chunky baseline 102.4-108.1 us across runs;
  measured variants taught that early DMA is PACKET-rate limited — one
  packet per partition line, 2 KB max — so sub-2KB lines and extra
  doorbells slow the head; SWDGE traffic steals ring throughput; and a
  PE idle gap > the HAM window re-throttles the clock to 1.2 GHz and
  costs ~3 us of cold matmuls):
    - w1 k-tiles 0-3 are shipped and consumed as e3m4 fp8 (4-bit
      mantissa, 1 cycle/row like bf16); k-tiles 4-7 stay bf16. Both
      halves pre-scaled by 128 on the host (lossless for bf16, centers
      e3m4's tiny exponent range) so one PSUM chain mixes them; the
      1/128 descale rides the silu input-scale and the FC2 output copy
      multiplier — zero extra ops. w1 drops 8 -> 6 MB. Simulated
      end-to-end rel-err 1.40e-2 vs the 2e-2 gate (bf16 baseline
      4.4e-3); HW matches the simulation to 4 digits.
    - Packet-efficient streaming: xt moves in 2-kt chunks (2 KB lines),
      w1a in >=4-slot groups (2 KB e3m4 lines, half the packet count of
      the bf16 equivalent), w1b in 2-slot groups (2 KB lines); pair-0
      criticals balanced across both HWDGE rings so its first pieces
      (xt[0:2] on sync, w1a[0:4] on scalar, 128 packets each) land
      ~10.6-12.3 us. GpSimd's slow SWDGE (~17 pkts/us) carries only
      xt[4:6], consumed last in pair-0's reordered chain.
    - Pair-0 runs proj/gate interleaved per k-tile in transfer-arrival
      order (0,1,2,3,6,7,4,5) so its ~3.5 us of PE work hides inside
      the DMA window; warmup matmuls fill the inter-arrival gaps.
    - PE warmup: matmuls on a memset tile flip the HAM clock-gate
      (1.2 -> 2.4 GHz needs ~3.4 us sustained busy) before the first
      real matmul; overshooting the bridge is ~free, undershooting
      resets the HAM window (+2.9 us measured). A dummy 8-element silu
      right after boot forces the SILU ACT_TABLE_LOAD during the DMA
      window.
    - PSUM pool = 8 x 1-bank [128,512] slots so 4 FC1 proj/gate pairs
      can be in flight; SwiGLU readout lags PE without blocking PSUM
      recycling. ACT runs only the 16 silus; FC2 PSUM->SBUF copies live
      on DVE; output DMA triggers on sync (idle after ~27 us).
    - Final FC2 m-tile splits into two independent half-column chains,
      and the very last half-chain drains via two parallel quarter
      copies (DVE + ACT) with doorbells on different engines.
"""

import time

import numpy as np
import ml_dtypes

import concourse.bass as bass
import concourse.bacc as bacc
import concourse.mybir as mybir
import concourse.tile as tile
from concourse.bass_utils import run_bass_kernel_spmd

NUM_TOKENS = 2048
HIDDEN = 1024
INTER = 2048
NUM_EXPERTS = 8
TOPK = 2
NCORES = 8
P = 128
KT1 = HIDDEN // P         # 8  k-tiles (FC1 contraction)
KTA = KT1 // 2            # 4  e3m4 k-tiles (kt 0-3)
MT1 = 2 * INTER // P      # 32 m-tiles (FC1 output rows = proj+gate)
MT1H = INTER // P         # 16 proj/gate pair count
KT2 = INTER // P          # 16 k-tiles (FC2 contraction)
MT2 = HIDDEN // P         # 8  m-tiles (FC2 output rows)
W1SCALE = 128.0           # host-side w1 pre-scale (both halves)

BF16 = mybir.dt.bfloat16
F8E3 = mybir.dt.float8e3
F32 = mybir.dt.float32
np_bf16 = ml_dtypes.bfloat16
np_e3m4 = ml_dtypes.float8_e3m4

# [0, 16, 1, 17, ...] — interleave proj/gate m-tiles into adjacent pairs
_W1_ORDER = np.arange(MT1).reshape(2, MT1H).T.reshape(-1)

NWARM_PRE = 9   # warmup matmuls before pair-0's first arrival (10.6-12.3
                # us depending on ring phase). Overshooting is ~free: early
                # real matmuls would run at the cold 1.2 GHz clock anyway,
                # so a warmup in their place costs nothing. Undershooting
                # (v7: 5) idles the PE, resets the HAM activity window, and
                # every matmul until ~18 us runs cold (+2.9 us measured).
NWARM_GAP = 2   # warmups interleaved after pair-0's kt1/kt3 groups to fill
                # the ~0.9-1.9 us inter-transfer arrival gaps so the PE
                # stays busy while pair-0 consumes transfers in arrival
                # order

# Pair-0's proj/gate chains run interleaved per k-tile in transfer-arrival
# order — xt moves in 2-kt chunks (2 KB lines, full packet efficiency):
# xt[0:2] + w1a[0:4] land first (~10.6 us), xt[2:4] (~12.5), xt[6:8]+b[0:2]
# (~14.2), and gpsimd's slow xt[4:6] last (~15.4) — so pair-0's 3.5 us of
# PE work hides inside the DMA window instead of following it.
_P0_KT_ORDER = (0, 1, 2, 3, 6, 7, 4, 5)

_graph_cache: dict = {}


def _build(NCH: int, CH: int) -> bass.Bass:
    """Per-core Bass graph for capacity C_pad = NCH * CH (CH <= 512)."""
    nc = bacc.Bacc("TRN2", target_bir_lowering=False, debug=False)

    xt_d = nc.declare_dram_parameter("xt", [P, KT1, NCH, CH], BF16, isOutput=False)
    # w1 split by contraction half: kt 0-3 e3m4, kt 4-7 bf16; slot axis
    # pair-interleaved (_W1_ORDER) so pair p's two m-tiles are adjacent.
    w1a_d = nc.declare_dram_parameter("w1a", [P, MT1, KTA, P], F8E3, isOutput=False)
    w1b_d = nc.declare_dram_parameter("w1b", [P, MT1, KTA, P], BF16, isOutput=False)
    w2_d = nc.declare_dram_parameter("w2", [P, MT2, KT2, P], BF16, isOutput=False)
    # bf16 output: halves the output DMA on the kernel tail; the host-side
    # combine upcasts to f32 (adds ~0.2% rounding — well within the gate).
    out_d = nc.declare_dram_parameter("out", [MT2, NCH, P, CH], BF16, isOutput=True)

    inv_scale = 1.0 / W1SCALE

    with tile.TileContext(nc) as tc:
        with (
            tc.tile_pool(name="weights", bufs=1) as wpool,
            tc.tile_pool(name="xin", bufs=1) as xpool,
            tc.tile_pool(name="actp", bufs=2) as apool,
            tc.tile_pool(name="tmp", bufs=4) as tpool,
            tc.tile_pool(name="osb", bufs=4) as opool,
            tc.tile_pool(name="psum", bufs=8, space="PSUM") as pspool,
        ):
            xt = xpool.tile([P, KT1, NCH, CH], BF16, tag="xt")
            w1a = wpool.tile([P, MT1, KTA, P], F8E3, tag="w1a")
            w1b = wpool.tile([P, MT1, KTA, P], BF16, tag="w1b")
            w2 = wpool.tile([P, MT2, KT2, P], BF16, tag="w2")
            dummy = xpool.tile([P, 640], BF16, tag="dummy")

            # Memset on DVE (idle at boot) so the warmup matmuls start as
            # early as possible.
            nc.vector.memset(dummy[:], 0.0)

            # PE warmup on the memset tile while inputs stream in.
            warm_ps = pspool.tile([P, 512], F32, tag="ps", name="warmps")

            def warm(n):
                for _ in range(n):
                    nc.tensor.matmul(
                        warm_ps[:, :], dummy[:, :128], dummy[:, 128:640],
                        start=True, stop=True,
                    )

            warm(NWARM_PRE)

            # Force the SILU activation-table load (~1.3 us, non-blocking)
            # right at engine boot, overlapped with the first input DMAs.
            tdum = tpool.tile([P, 8], F32, tag="tmp", name="tdum")
            nc.scalar.activation(
                tdum[:], dummy[:, :8], mybir.ActivationFunctionType.Silu
            )

            # ---- input DMA triggers ----
            # Early DMA is packet-rate limited: one packet per partition
            # line, 2 KB max per packet. w1a therefore moves in >=4-slot
            # groups (2 KB e3m4 lines) so its packet count is half of the
            # bf16 equivalent. Pair-0 criticals are balanced at ~320
            # packets per HWDGE ring (the measured floor lands ~14.1 us);
            # gpsimd's slow SWDGE carries only xt[3:5], consumed late in
            # pair-0's reordered chain. Everything else streams in
            # consumption order, alternating rings.
            nc.sync.dma_start(out=xt[:, 0:2, 0], in_=xt_d[:, 0:2, 0])
            nc.scalar.dma_start(out=w1a[:, 0:4], in_=w1a_d[:, 0:4])
            nc.gpsimd.dma_start(out=xt[:, 4:6, 0], in_=xt_d[:, 4:6, 0])
            nc.sync.dma_start(out=xt[:, 2:4, 0], in_=xt_d[:, 2:4, 0])
            nc.scalar.dma_start(out=w1b[:, 0:2], in_=w1b_d[:, 0:2])
            nc.scalar.dma_start(out=xt[:, 6:8, 0], in_=xt_d[:, 6:8, 0])
            nc.sync.dma_start(out=w1b[:, 2:4], in_=w1b_d[:, 2:4])
            nc.scalar.dma_start(out=w1a[:, 4:8], in_=w1a_d[:, 4:8])
            nc.sync.dma_start(out=w1b[:, 4:6], in_=w1b_d[:, 4:6])
            nc.scalar.dma_start(out=w1b[:, 6:8], in_=w1b_d[:, 6:8])
            nc.sync.dma_start(out=w1a[:, 8:16], in_=w1a_d[:, 8:16])
            nc.scalar.dma_start(out=w1b[:, 8:12], in_=w1b_d[:, 8:12])
            nc.sync.dma_start(out=w1b[:, 12:16], in_=w1b_d[:, 12:16])
            nc.scalar.dma_start(out=w1a[:, 16:24], in_=w1a_d[:, 16:24])
            nc.sync.dma_start(out=w1b[:, 16:20], in_=w1b_d[:, 16:20])
            nc.scalar.dma_start(out=w1b[:, 20:24], in_=w1b_d[:, 20:24])
            nc.sync.dma_start(out=w1a[:, 24:32], in_=w1a_d[:, 24:32])
            nc.scalar.dma_start(out=w1b[:, 24:28], in_=w1b_d[:, 24:28])
            nc.sync.dma_start(out=w1b[:, 28:32], in_=w1b_d[:, 28:32])
            nc.scalar.dma_start(out=w2[:, MT2 // 2 :], in_=w2_d[:, MT2 // 2 :])
            nc.sync.dma_start(out=w2[:, : MT2 // 2], in_=w2_d[:, : MT2 // 2])
            for ci in range(1, NCH):
                nc.sync.dma_start(out=xt[:, :, ci], in_=xt_d[:, :, ci])

            def lhsT1(j, kt):
                """FC1 stationary operand for slot j (pair-ordered), k-tile kt."""
                if kt < KTA:
                    return w1a[:, j, kt, :]
                return w1b[:, j, kt - KTA, :]

            for ci in range(NCH):
                # ---- FC1 (proj/gate pair per iteration) + SwiGLU ----
                act = apool.tile([P, KT2, CH], BF16, tag="act", name=f"act{ci}")
                for mt in range(MT1H):
                    ps_p = pspool.tile([P, 512], F32, tag="ps", name=f"psp{ci}_{mt}")
                    ps_g = pspool.tile([P, 512], F32, tag="ps", name=f"psg{ci}_{mt}")
                    if ci == 0 and mt == 0:
                        # Pair 0: proj/gate interleaved per k-tile in DMA
                        # arrival order so every landing transfer feeds the
                        # PE immediately; warmups fill the arrival gaps.
                        for i, kt in enumerate(_P0_KT_ORDER):
                            for ps, pg in ((ps_p, 0), (ps_g, 1)):
                                nc.tensor.matmul(
                                    ps[:, :CH],
                                    lhsT1(pg, kt),
                                    xt[:, kt, 0, :],
                                    start=(i == 0),
                                    stop=(i == KT1 - 1),
                                )
                            if kt in (1, 3):
                                warm(NWARM_GAP)
                    else:
                        for ps, pg in ((ps_p, 0), (ps_g, 1)):
                            for kt in range(KT1):
                                nc.tensor.matmul(
                                    ps[:, :CH],
                                    lhsT1(2 * mt + pg, kt),
                                    xt[:, kt, ci, :],
                                    start=(kt == 0),
                                    stop=(kt == KT1 - 1),
                                )
                    tmp = tpool.tile([P, CH], F32, tag="tmp", name=f"tmp{ci}_{mt}")
                    # PSUM carries 128*fc1 — the silu input-scale descales.
                    nc.scalar.activation(
                        tmp[:], ps_p[:, :CH], mybir.ActivationFunctionType.Silu,
                        scale=inv_scale,
                    )
                    nc.vector.tensor_mul(act[:, mt], tmp[:], ps_g[:, :CH])

                # ---- FC2 ----
                # act carries 128*act_true; the final copies descale by
                # 1/128. Copies live on DVE; output DMA triggers on sync
                # (idle once input streaming finishes ~27 us).
                for m2 in range(MT2):
                    ps_o = pspool.tile([P, 512], F32, tag="ps", name=f"pso{ci}_{m2}")
                    o_sb = opool.tile([P, CH], BF16, tag="o", name=f"osb{ci}_{m2}")
                    if ci == NCH - 1 and m2 == MT2 - 1:
                        # Final m-tile: two independent half-column chains so
                        # the first half drains (copy+DMA) while the PE runs
                        # the second half. The very last half drains as two
                        # parallel quarter copies (DVE + ACT) with doorbells
                        # on different engines to shorten the post-last-
                        # matmul tail. Separate PSUM tiles: tile-granular
                        # WAR tracking would otherwise stall chain B behind
                        # chain A's copy.
                        ps_b = pspool.tile([P, 512], F32, tag="ps", name="psoB")
                        h = CH // 2
                        q = CH // 4
                        for ps, (c0, c1) in ((ps_o, (0, h)), (ps_b, (h, CH))):
                            for kt2 in range(KT2):
                                nc.tensor.matmul(
                                    ps[:, c0:c1],
                                    w2[:, m2, kt2, :],
                                    act[:, kt2, c0:c1],
                                    start=(kt2 == 0),
                                    stop=(kt2 == KT2 - 1),
                                )
                            if c1 == CH:
                                # Last half: two parallel quarter drains.
                                nc.vector.tensor_scalar_mul(
                                    o_sb[:, c0 : c0 + q], ps[:, c0 : c0 + q],
                                    inv_scale,
                                )
                                nc.scalar.activation(
                                    o_sb[:, c0 + q : c1], ps[:, c0 + q : c1],
                                    mybir.ActivationFunctionType.Copy,
                                    scale=inv_scale,
                                )
                                nc.sync.dma_start(
                                    out=out_d[m2, ci, :, c0 : c0 + q],
                                    in_=o_sb[:, c0 : c0 + q],
                                )
                                nc.scalar.dma_start(
                                    out=out_d[m2, ci, :, c0 + q : c1],
                                    in_=o_sb[:, c0 + q : c1],
                                )
                            else:
                                nc.vector.tensor_scalar_mul(
                                    o_sb[:, c0:c1], ps[:, c0:c1], inv_scale
                                )
                                nc.sync.dma_start(
                                    out=out_d[m2, ci, :, c0:c1],
                                    in_=o_sb[:, c0:c1],
                                )
                    else:
                        for kt2 in range(KT2):
                            nc.tensor.matmul(
                                ps_o[:, :CH],
                                w2[:, m2, kt2, :],
                                act[:, kt2, :],
                                start=(kt2 == 0),
                                stop=(kt2 == KT2 - 1),
                            )
                        nc.vector.tensor_scalar_mul(o_sb[:], ps_o[:, :CH], inv_scale)
                        nc.sync.dma_start(out=out_d[m2, ci], in_=o_sb[:])

    nc.compile()
    return nc


def _get_graph(NCH: int, CH: int) -> bass.Bass:
    key = (NCH, CH)
    if key not in _graph_cache:
        _graph_cache[key] = _build(NCH, CH)
    return _graph_cache[key]


def _route(router_logits: np.ndarray):
    """Top-2 + softmax, exactly matching jax.lax.top_k tie-breaking."""
    idx = np.argsort(-router_logits, axis=-1, kind="stable")[:, :TOPK]
    tl = np.take_along_axis(router_logits, idx, axis=-1)
    ex = np.exp(tl - tl.max(-1, keepdims=True))
    sc = (ex / ex.sum(-1, keepdims=True)).astype(np.float32)
    return idx, sc


def run(hidden_states, router_logits, w1, w2, trace=False, trace_kwargs=None):
    hs = np.asarray(hidden_states, dtype=np.float32)
    rl = np.asarray(router_logits, dtype=np.float32)
    w1 = np.asarray(w1, dtype=np.float32)
    w2 = np.asarray(w2, dtype=np.float32)
    N, D = hs.shape

    idx, sc = _route(rl)

    tok_lists = []
    for e in range(NUM_EXPERTS):
        toks, slots = np.nonzero(idx == e)
        tok_lists.append((toks, slots))
    cmax = max(len(t) for t, _ in tok_lists)

    # Full-width (N=512) matmuls stream ~5% fewer PE cycles than two ragged
    # chunks. When the capacity overhang past a 512 multiple is small, cap
    # the device capacity at the multiple and run the few overflow tokens
    # through a f32 numpy epilogue on the host (<= 64 rows per expert;
    # routing/combine already live there).
    if cmax > 512 and cmax % 512 <= 64:
        C_dev = 512 * (cmax // 512)
    else:
        C_dev = cmax
    NCH = max(1, -(-C_dev // 512))
    CH = -(-C_dev // (NCH * 2)) * 2  # chunk width, multiple of 2
    C_pad = CH * NCH

    in_maps = []
    for e in range(NUM_EXPERTS):
        toks = tok_lists[e][0][:C_pad]
        x = np.zeros((C_pad, D), np.float32)
        x[: len(toks)] = hs[toks]
        xt = x.T.reshape(KT1, P, NCH, CH).transpose(1, 0, 2, 3).astype(np_bf16)
        # w1 pre-scaled by 128; slot axis pair-interleaved (_W1_ORDER);
        # kt 0-3 e3m4 (w1a), kt 4-7 bf16 (w1b), both slot-major.
        w1s = w1[e] * W1SCALE
        a = w1s[: KTA * P].reshape(KTA, P, MT1, P).transpose(1, 2, 0, 3)
        w1ae = np.ascontiguousarray(a[:, _W1_ORDER]).astype(np_e3m4)
        b = w1s[KTA * P :].reshape(KTA, P, MT1, P).transpose(1, 2, 0, 3)
        w1be = np.ascontiguousarray(b[:, _W1_ORDER]).astype(np_bf16)
        w2e = w2[e].reshape(KT2, P, MT2, P).transpose(1, 2, 0, 3).astype(np_bf16)
        in_maps.append({"xt": xt, "w1a": w1ae, "w1b": w1be, "w2": w2e})

    nc = _get_graph(NCH, CH)

    res = None
    for attempt in range(4):
        try:
            res = run_bass_kernel_spmd(
                nc,
                in_maps,
                core_ids=list(range(NCORES)),
                trace=trace,
                **(trace_kwargs or {}),
            )
            break
        except Exception:
            if attempt == 3:
                raise
            time.sleep(15 * (attempt + 1))

    out = np.zeros((N, D), np.float32)
    for e in range(NUM_EXPERTS):
        toks, slots = tok_lists[e]
        n_dev = min(len(toks), C_pad)
        oT = np.asarray(res.results[e]["out"]).astype(np.float32)
        oT = oT.transpose(0, 2, 1, 3).reshape(HIDDEN, C_pad)
        out[toks[:n_dev]] += sc[toks[:n_dev], slots[:n_dev]][:, None] * oT[:, :n_dev].T
        if n_dev < len(toks):
            # f32 host epilogue for the few overflow tokens past capacity
            ot, osl = toks[n_dev:], slots[n_dev:]
            h = hs[ot] @ w1[e]
            proj, gate = h[:, :INTER], h[:, INTER:]
            o = (proj / (1.0 + np.exp(-proj)) * gate) @ w2[e]
            out[ot] += sc[ot, osl][:, None] * o
    return out, res


def kernel(hidden_states, router_logits, w1, w2):
    out, _ = run(hidden_states, router_logits, w1, w2)
    return out


# revision 19
# speedup vs baseline: 1.0127x; 1.0127x over previous
"""MoE (AriaExperts) Trainium2 kernel — expert parallelism across 8 NeuronCores.

Strategy:
  - Host: top-2 routing + softmax over [2048, 8] logits (tiny), build the
    per-expert token batches (the "all-to-all" is realized at input
    distribution time), and the weighted scatter-add combine at the end.
  - Device (SPMD, 1 expert per core): dense GEMM chain in transposed
    activation layout so both matmuls consume the expert weights directly
    as the stationary (lhsT) operand with zero on-device transposes:
        H^T  = W1^T-tiles @ X^T      [2*INTER, C]
        actT = silu(projT) * gateT   [INTER, C]
        outT = W2-tiles   @ actT     [HIDDEN, C]
    All matmul dtypes are 1-cycle/row; f32 PSUM accumulation.

  Trace-driven design (bf16 chunky baseline 102.4-108.1 us across runs;
  measured variants taught that early DMA is PACKET-rate limited — one
  packet per partition line, 2 KB max — so sub-2KB lines and extra
  doorbells slow the head; SWDGE traffic steals ring throughput; and a
  PE idle gap > the HAM window re-throttles the clock to 1.2 GHz and
  costs ~3 us of cold matmuls):
    - w1 k-tiles 0-3 are shipped and consumed as e3m4 fp8 (4-bit
      mantissa, 1 cycle/row like bf16); k-tiles 4-7 stay bf16. Both
      halves pre-scaled by 128 on the host (lossless for bf16, centers
      e3m4's tiny exponent range) so one PSUM chain mixes them; the
      1/128 descale rides the silu input-scale and the FC2 output copy
      multiplier — zero extra ops. w1 drops 8 -> 6 MB. Simulated
      end-to-end rel-err 1.40e-2 vs the 2e-2 gate (bf16 baseline
      4.4e-3); HW matches the simulation to 4 digits.
    - Packet-efficient streaming: xt moves in 2-kt chunks (2 KB lines),
      w1a in >=4-slot groups (2 KB e3m4 lines, half the packet count of
      the bf16 equivalent), w1b in 2-slot groups (2 KB lines); pair-0
      criticals balanced across both HWDGE rings so its first pieces
      (xt[0:2] on sync, w1a[0:4] on scalar, 128 packets each) land
      ~10.6-12.3 us. GpSimd's slow SWDGE (~17 pkts/us) carries only
      xt[4:6], consumed last in pair-0's reordered chain.
    - Pair-0 runs proj/gate interleaved per k-tile in transfer-arrival
      order (0,1,2,3,6,7,4,5) so its ~3.5 us of PE work hides inside
      the DMA window; warmup matmuls fill the inter-arrival gaps.
    - PE warmup: matmuls on a memset tile flip the HAM clock-gate
      (1.2 -> 2.4 GHz needs ~3.4 us sustained busy) before the first
      real matmul; overshooting the bridge is ~free, undershooting
      resets the HAM window (+2.9 us measured). A dummy 8-element silu
      right after boot forces the SILU ACT_TABLE_LOAD during the DMA
      window.
    - PSUM pool = 8 x 1-bank [128,512] slots so 4 FC1 proj/gate pairs
      can be in flight; SwiGLU readout lags PE without blocking PSUM
      recycling. ACT runs only the 16 silus; FC2 PSUM->SBUF copies live
      on DVE; output DMA triggers on sync (idle after ~27 us).
    - Final FC2 m-tile splits into two independent half-column chains,
      and the very last half-chain drains via two parallel quarter
      copies (DVE + ACT) with doorbells on different engines.
"""

import time

import numpy as np
import ml_dtypes

import concourse.bass as bass
import concourse.bacc as bacc
import concourse.mybir as mybir
import concourse.tile as tile
from concourse.bass_utils import run_bass_kernel_spmd

NUM_TOKENS = 2048
HIDDEN = 1024
INTER = 2048
NUM_EXPERTS = 8
TOPK = 2
NCORES = 8
P = 128
KT1 = HIDDEN // P         # 8  k-tiles (FC1 contraction)
KTA = KT1 // 2            # 4  e3m4 k-tiles (kt 0-3)
MT1 = 2 * INTER // P      # 32 m-tiles (FC1 output rows = proj+gate)
MT1H = INTER // P         # 16 proj/gate pair count
KT2 = INTER // P          # 16 k-tiles (FC2 contraction)
MT2 = HIDDEN // P         # 8  m-tiles (FC2 output rows)
W1SCALE = 128.0           # host-side w1 pre-scale (both halves)

BF16 = mybir.dt.bfloat16
F8E3 = mybir.dt.float8e3
F32 = mybir.dt.float32
np_bf16 = ml_dtypes.bfloat16
np_e3m4 = ml_dtypes.float8_e3m4

# [0, 16, 1, 17, ...] — interleave proj/gate m-tiles into adjacent pairs
_W1_ORDER = np.arange(MT1).reshape(2, MT1H).T.reshape(-1)

NWARM_PRE = 9   # warmup matmuls before pair-0's first arrival (10.6-12.3
                # us depending on ring phase). Overshooting is ~free: early
                # real matmuls would run at the cold 1.2 GHz clock anyway,
                # so a warmup in their place costs nothing. Undershooting
                # (v7: 5) idles the PE, resets the HAM activity window, and
                # every matmul until ~18 us runs cold (+2.9 us measured).
NWARM_GAP = 2   # warmups interleaved after pair-0's kt1/kt3 groups to fill
                # the ~0.9-1.9 us inter-transfer arrival gaps so the PE
                # stays busy while pair-0 consumes transfers in arrival
                # order

# Pairs 0-1 run co-scheduled, proj/gate interleaved per k-tile in
# transfer-arrival order — xt moves in 2-kt chunks (2 KB lines, full
# packet efficiency): xt[0:2] + w1a[0:4] (which covers BOTH pairs' e3m4
# slots) land first (~10.6-12.3 us), xt[2:4] + w1b[0:2] next, then
# xt[6:8] + w1b[2:4], and gpsimd's slow xt[4:6] last — so each arrival
# event feeds 8 real matmuls and ~7 us of FC1 work hides inside the DMA
# window instead of following it.
_P01_KT_BLOCKS = ((0, 1), (2, 3), (6, 7), (4, 5))

_graph_cache: dict = {}


def _build(NCH: int, CH: int) -> bass.Bass:
    """Per-core Bass graph for capacity C_pad = NCH * CH (CH <= 512)."""
    nc = bacc.Bacc("TRN2", target_bir_lowering=False, debug=False)

    xt_d = nc.declare_dram_parameter("xt", [P, KT1, NCH, CH], BF16, isOutput=False)
    # w1 split by contraction half: kt 0-3 e3m4, kt 4-7 bf16; slot axis
    # pair-interleaved (_W1_ORDER) so pair p's two m-tiles are adjacent.
    w1a_d = nc.declare_dram_parameter("w1a", [P, MT1, KTA, P], F8E3, isOutput=False)
    w1b_d = nc.declare_dram_parameter("w1b", [P, MT1, KTA, P], BF16, isOutput=False)
    w2_d = nc.declare_dram_parameter("w2", [P, MT2, KT2, P], BF16, isOutput=False)
    # bf16 output: halves the output DMA on the kernel tail; the host-side
    # combine upcasts to f32 (adds ~0.2% rounding — well within the gate).
    out_d = nc.declare_dram_parameter("out", [MT2, NCH, P, CH], BF16, isOutput=True)

    inv_scale = 1.0 / W1SCALE

    with tile.TileContext(nc) as tc:
        with (
            tc.tile_pool(name="weights", bufs=1) as wpool,
            tc.tile_pool(name="xin", bufs=1) as xpool,
            tc.tile_pool(name="actp", bufs=2) as apool,
            tc.tile_pool(name="tmp", bufs=4) as tpool,
            tc.tile_pool(name="osb", bufs=4) as opool,
            tc.tile_pool(name="psum", bufs=8, space="PSUM") as pspool,
        ):
            xt = xpool.tile([P, KT1, NCH, CH], BF16, tag="xt")
            w1a = wpool.tile([P, MT1, KTA, P], F8E3, tag="w1a")
            w1b = wpool.tile([P, MT1, KTA, P], BF16, tag="w1b")
            w2 = wpool.tile([P, MT2, KT2, P], BF16, tag="w2")
            dummy = xpool.tile([P, 640], BF16, tag="dummy")

            # Memset on DVE (idle at boot) so the warmup matmuls start as
            # early as possible.
            nc.vector.memset(dummy[:], 0.0)

            # PE warmup on the memset tile while inputs stream in.
            warm_ps = pspool.tile([P, 512], F32, tag="ps", name="warmps")

            def warm(n):
                for _ in range(n):
                    nc.tensor.matmul(
                        warm_ps[:, :], dummy[:, :128], dummy[:, 128:640],
                        start=True, stop=True,
                    )

            warm(NWARM_PRE)

            # Force the SILU activation-table load (~1.3 us, non-blocking)
            # right at engine boot, overlapped with the first input DMAs.
            tdum = tpool.tile([P, 8], F32, tag="tmp", name="tdum")
            nc.scalar.activation(
                tdum[:], dummy[:, :8], mybir.ActivationFunctionType.Silu
            )

            # ---- input DMA triggers ----
            # Early DMA is packet-rate limited: one packet per partition
            # line, 2 KB max per packet. w1a therefore moves in >=4-slot
            # groups (2 KB e3m4 lines) so its packet count is half of the
            # bf16 equivalent. Pair-0 criticals are balanced at ~320
            # packets per HWDGE ring (the measured floor lands ~14.1 us);
            # gpsimd's slow SWDGE carries only xt[3:5], consumed late in
            # pair-0's reordered chain. Everything else streams in
            # consumption order, alternating rings.
            nc.sync.dma_start(out=xt[:, 0:2, 0], in_=xt_d[:, 0:2, 0])
            nc.scalar.dma_start(out=w1a[:, 0:4], in_=w1a_d[:, 0:4])
            nc.gpsimd.dma_start(out=xt[:, 4:6, 0], in_=xt_d[:, 4:6, 0])
            nc.sync.dma_start(out=xt[:, 2:4, 0], in_=xt_d[:, 2:4, 0])
            nc.scalar.dma_start(out=w1b[:, 0:2], in_=w1b_d[:, 0:2])
            nc.scalar.dma_start(out=xt[:, 6:8, 0], in_=xt_d[:, 6:8, 0])
            nc.sync.dma_start(out=w1b[:, 2:4], in_=w1b_d[:, 2:4])
            nc.scalar.dma_start(out=w1a[:, 4:8], in_=w1a_d[:, 4:8])
            nc.sync.dma_start(out=w1b[:, 4:6], in_=w1b_d[:, 4:6])
            nc.scalar.dma_start(out=w1b[:, 6:8], in_=w1b_d[:, 6:8])
            nc.sync.dma_start(out=w1a[:, 8:16], in_=w1a_d[:, 8:16])
            nc.scalar.dma_start(out=w1b[:, 8:12], in_=w1b_d[:, 8:12])
            nc.sync.dma_start(out=w1b[:, 12:16], in_=w1b_d[:, 12:16])
            nc.scalar.dma_start(out=w1a[:, 16:24], in_=w1a_d[:, 16:24])
            nc.sync.dma_start(out=w1b[:, 16:20], in_=w1b_d[:, 16:20])
            nc.scalar.dma_start(out=w1b[:, 20:24], in_=w1b_d[:, 20:24])
            nc.sync.dma_start(out=w1a[:, 24:32], in_=w1a_d[:, 24:32])
            nc.scalar.dma_start(out=w1b[:, 24:28], in_=w1b_d[:, 24:28])
            nc.sync.dma_start(out=w1b[:, 28:32], in_=w1b_d[:, 28:32])
            nc.scalar.dma_start(out=w2[:, MT2 // 2 :], in_=w2_d[:, MT2 // 2 :])
            nc.sync.dma_start(out=w2[:, : MT2 // 2], in_=w2_d[:, : MT2 // 2])
            for ci in range(1, NCH):
                nc.sync.dma_start(out=xt[:, :, ci], in_=xt_d[:, :, ci])

            def lhsT1(j, kt):
                """FC1 stationary operand for slot j (pair-ordered), k-tile kt."""
                if kt < KTA:
                    return w1a[:, j, kt, :]
                return w1b[:, j, kt - KTA, :]

            for ci in range(NCH):
                # ---- FC1 (proj/gate pair per iteration) + SwiGLU ----
                act = apool.tile([P, KT2, CH], BF16, tag="act", name=f"act{ci}")

                def swiglu(mt, ps_p, ps_g):
                    tmp = tpool.tile([P, CH], F32, tag="tmp", name=f"tmp{ci}_{mt}")
                    # PSUM carries 128*fc1 — the silu input-scale descales.
                    nc.scalar.activation(
                        tmp[:], ps_p[:, :CH], mybir.ActivationFunctionType.Silu,
                        scale=inv_scale,
                    )
                    nc.vector.tensor_mul(act[:, mt], tmp[:], ps_g[:, :CH])

                mt0 = 0
                if ci == 0:
                    # Pairs 0-1 co-scheduled in DMA arrival order: per
                    # k-tile block, both pairs' proj/gate matmuls run so
                    # every landing transfer feeds 8 matmuls immediately;
                    # a couple of warmups cover a late second transfer.
                    mt0 = 2
                    chains = []
                    for mt in range(2):
                        ps_p = pspool.tile([P, 512], F32, tag="ps", name=f"psp0_{mt}")
                        ps_g = pspool.tile([P, 512], F32, tag="ps", name=f"psg0_{mt}")
                        chains.append((mt, ps_p, ps_g))
                    for bi, kts in enumerate(_P01_KT_BLOCKS):
                        for kt in kts:
                            for mt, ps_p, ps_g in chains:
                                for ps, pg in ((ps_p, 0), (ps_g, 1)):
                                    nc.tensor.matmul(
                                        ps[:, :CH],
                                        lhsT1(2 * mt + pg, kt),
                                        xt[:, kt, 0, :],
                                        start=(kt == 0),
                                        stop=(kt == 5),
                                    )
                        if bi == 0:
                            warm(NWARM_GAP)
                    for mt, ps_p, ps_g in chains:
                        swiglu(mt, ps_p, ps_g)

                for mt in range(mt0, MT1H):
                    ps_p = pspool.tile([P, 512], F32, tag="ps", name=f"psp{ci}_{mt}")
                    ps_g = pspool.tile([P, 512], F32, tag="ps", name=f"psg{ci}_{mt}")
                    for ps, pg in ((ps_p, 0), (ps_g, 1)):
                        for kt in range(KT1):
                            nc.tensor.matmul(
                                ps[:, :CH],
                                lhsT1(2 * mt + pg, kt),
                                xt[:, kt, ci, :],
                                start=(kt == 0),
                                stop=(kt == KT1 - 1),
                            )
                    swiglu(mt, ps_p, ps_g)

                # ---- FC2 ----
                # act carries 128*act_true; the final copies descale by
                # 1/128. Copies live on DVE; output DMA triggers on sync
                # (idle once input streaming finishes ~27 us).
                for m2 in range(MT2):
                    ps_o = pspool.tile([P, 512], F32, tag="ps", name=f"pso{ci}_{m2}")
                    o_sb = opool.tile([P, CH], BF16, tag="o", name=f"osb{ci}_{m2}")
                    if ci == NCH - 1 and m2 == MT2 - 1:
                        # Final m-tile: two independent half-column chains so
                        # the first half drains (copy+DMA) while the PE runs
                        # the second half. The very last half drains as two
                        # parallel quarter copies (DVE + ACT) with doorbells
                        # on different engines to shorten the post-last-
                        # matmul tail. Separate PSUM tiles: tile-granular
                        # WAR tracking would otherwise stall chain B behind
                        # chain A's copy.
                        ps_b = pspool.tile([P, 512], F32, tag="ps", name="psoB")
                        h = CH // 2
                        q = CH // 4
                        for ps, (c0, c1) in ((ps_o, (0, h)), (ps_b, (h, CH))):
                            for kt2 in range(KT2):
                                nc.tensor.matmul(
                                    ps[:, c0:c1],
                                    w2[:, m2, kt2, :],
                                    act[:, kt2, c0:c1],
                                    start=(kt2 == 0),
                                    stop=(kt2 == KT2 - 1),
                                )
                            if c1 == CH:
                                # Last half: parallel partition-split drain
                                # (DVE + ACT copies, doorbells on different
                                # engines; 64-line transfers halve the
                                # packet count of each final DMA flight).
                                nc.vector.tensor_scalar_mul(
                                    o_sb[: P // 2, c0:c1], ps[: P // 2, c0:c1],
                                    inv_scale,
                                )
                                nc.scalar.activation(
                                    o_sb[P // 2 :, c0:c1], ps[P // 2 :, c0:c1],
                                    mybir.ActivationFunctionType.Copy,
                                    scale=inv_scale,
                                )
                                nc.sync.dma_start(
                                    out=out_d[m2, ci, : P // 2, c0:c1],
                                    in_=o_sb[: P // 2, c0:c1],
                                )
                                nc.scalar.dma_start(
                                    out=out_d[m2, ci, P // 2 :, c0:c1],
                                    in_=o_sb[P // 2 :, c0:c1],
                                )
                            else:
                                nc.vector.tensor_scalar_mul(
                                    o_sb[:, c0:c1], ps[:, c0:c1], inv_scale
                                )
                                nc.sync.dma_start(
                                    out=out_d[m2, ci, :, c0:c1],
                                    in_=o_sb[:, c0:c1],
                                )
                    else:
                        for kt2 in range(KT2):
                            nc.tensor.matmul(
                                ps_o[:, :CH],
                                w2[:, m2, kt2, :],
                                act[:, kt2, :],
                                start=(kt2 == 0),
                                stop=(kt2 == KT2 - 1),
                            )
                        nc.vector.tensor_scalar_mul(o_sb[:], ps_o[:, :CH], inv_scale)
                        nc.sync.dma_start(out=out_d[m2, ci], in_=o_sb[:])

    nc.compile()
    return nc


def _get_graph(NCH: int, CH: int) -> bass.Bass:
    key = (NCH, CH)
    if key not in _graph_cache:
        _graph_cache[key] = _build(NCH, CH)
    return _graph_cache[key]


def _route(router_logits: np.ndarray):
    """Top-2 + softmax, exactly matching jax.lax.top_k tie-breaking."""
    idx = np.argsort(-router_logits, axis=-1, kind="stable")[:, :TOPK]
    tl = np.take_along_axis(router_logits, idx, axis=-1)
    ex = np.exp(tl - tl.max(-1, keepdims=True))
    sc = (ex / ex.sum(-1, keepdims=True)).astype(np.float32)
    return idx, sc


def run(hidden_states, router_logits, w1, w2, trace=False, trace_kwargs=None):
    hs = np.asarray(hidden_states, dtype=np.float32)
    rl = np.asarray(router_logits, dtype=np.float32)
    w1 = np.asarray(w1, dtype=np.float32)
    w2 = np.asarray(w2, dtype=np.float32)
    N, D = hs.shape

    idx, sc = _route(rl)

    tok_lists = []
    for e in range(NUM_EXPERTS):
        toks, slots = np.nonzero(idx == e)
        tok_lists.append((toks, slots))
    cmax = max(len(t) for t, _ in tok_lists)

    # Full-width (N=512) matmuls stream ~5% fewer PE cycles than two ragged
    # chunks. When the capacity overhang past a 512 multiple is small, cap
    # the device capacity at the multiple and run the few overflow tokens
    # through a f32 numpy epilogue on the host (<= 64 rows per expert;
    # routing/combine already live there).
    if cmax > 512 and cmax % 512 <= 64:
        C_dev = 512 * (cmax // 512)
    else:
        C_dev = cmax
    NCH = max(1, -(-C_dev // 512))
    CH = -(-C_dev // (NCH * 2)) * 2  # chunk width, multiple of 2
    C_pad = CH * NCH

    in_maps = []
    for e in range(NUM_EXPERTS):
        toks = tok_lists[e][0][:C_pad]
        x = np.zeros((C_pad, D), np.float32)
        x[: len(toks)] = hs[toks]
        xt = x.T.reshape(KT1, P, NCH, CH).transpose(1, 0, 2, 3).astype(np_bf16)
        # w1 pre-scaled by 128; slot axis pair-interleaved (_W1_ORDER);
        # kt 0-3 e3m4 (w1a), kt 4-7 bf16 (w1b), both slot-major.
        w1s = w1[e] * W1SCALE
        a = w1s[: KTA * P].reshape(KTA, P, MT1, P).transpose(1, 2, 0, 3)
        w1ae = np.ascontiguousarray(a[:, _W1_ORDER]).astype(np_e3m4)
        b = w1s[KTA * P :].reshape(KTA, P, MT1, P).transpose(1, 2, 0, 3)
        w1be = np.ascontiguousarray(b[:, _W1_ORDER]).astype(np_bf16)
        w2e = w2[e].reshape(KT2, P, MT2, P).transpose(1, 2, 0, 3).astype(np_bf16)
        in_maps.append({"xt": xt, "w1a": w1ae, "w1b": w1be, "w2": w2e})

    nc = _get_graph(NCH, CH)

    res = None
    for attempt in range(4):
        try:
            res = run_bass_kernel_spmd(
                nc,
                in_maps,
                core_ids=list(range(NCORES)),
                trace=trace,
                **(trace_kwargs or {}),
            )
            break
        except Exception:
            if attempt == 3:
                raise
            time.sleep(15 * (attempt + 1))

    out = np.zeros((N, D), np.float32)
    for e in range(NUM_EXPERTS):
        toks, slots = tok_lists[e]
        n_dev = min(len(toks), C_pad)
        oT = np.asarray(res.results[e]["out"]).astype(np.float32)
        oT = oT.transpose(0, 2, 1, 3).reshape(HIDDEN, C_pad)
        out[toks[:n_dev]] += sc[toks[:n_dev], slots[:n_dev]][:, None] * oT[:, :n_dev].T
        if n_dev < len(toks):
            # f32 host epilogue for the few overflow tokens past capacity
            ot, osl = toks[n_dev:], slots[n_dev:]
            h = hs[ot] @ w1[e]
            proj, gate = h[:, :INTER], h[:, INTER:]
            o = (proj / (1.0 + np.exp(-proj)) * gate) @ w2[e]
            out[ot] += sc[ot, osl][:, None] * o
    return out, res


def kernel(hidden_states, router_logits, w1, w2):
    out, _ = run(hidden_states, router_logits, w1, w2)
    return out
